# revision 22
# baseline (speedup 1.0000x reference)
"""Chamfer loss (single-direction) Trainium2 Bass kernel.

Problem: pc_src [B=4, 3, M=8192], pc_dst [B=4, 3, N=8192] (fp32).
  d2[b,m,n] = ||src[b,:,m] - dst[b,:,n]||^2
  out = mean over (b,m) of sqrt(min_n d2[b,m,n])

Sharding: 8 cores = 4 batches x 2 M-halves. Each core handles one batch's
dst [3, 8192] and a 4096-point slice of that batch's src. The min over n is
complete per core; the host concatenates per-core min-d2 vectors and does
the (tiny, O(B*M)) sqrt + mean.

Device algorithm per core -- plain fp8e5m2 multi-level augmented matmul with
the contraction dim zero-padded to K=128:
  Each fp32 value is decomposed into NLEV=6 e5m2 levels x = sum_a x_a with
  |x_a| shrinking ~8x per level, so the reconstruction residual is ~2^-18
  relative. fp8 x fp8 products are exact in the fp32 PSUM accumulator.

  d2 = ||s||^2 + ||d||^2 - 2 s.d expands over level pairs; all pairs with
  a + b <= LMAX=6 are kept (dropped terms ~8^-7):
    cross rows:  lhsT = -2*s_a[coord],  rhs = d_b[coord]   (26 pairs x 3)
    norm rows:   lhsT = ssq_a, rhs = 1  /  lhsT = 1, rhs = dsq_b  (6 + 6)
  = 90 contraction rows, zero-padded to 128. K <= 128 is free on the PE
  (throughput is 1 moving column/cycle regardless of K), and full-K
  matmuls keep the PE_HAM activity monitor warm: partial-K matmuls
  (45 or 90 rows) leave the PE clock-gated at 1.2 GHz (427 ns / 512-col
  matmul) forever, while K=128 un-throttles to 2.4 GHz (216 ns) after
  ~3.5 us. A burst of warm-up matmuls on scratch data runs while the
  input DMAs land so the real stream starts warm.

  The min-reduce runs on the VectorEngine with a custom pair-min op,
  one instruction per pair of [128, 1024] PSUM tiles:
    mins4[:, mt, pr] = min(BIG, min_free(min(psumA, sbufB)))
  (2 distance elements per cycle per lane, both read ports). ScalarE
  (otherwise idle) stages psumB into SBUF -- the ISA allows only one
  non-scalar DVE input in PSUM. The 4 pair results per M-tile land in
  independent accum slots (no serial chain), and one final grouped
  tensor_reduce collapses mins4 [P, m_tiles, 4] -> [P, m_tiles].
"""

import ml_dtypes
import numpy as np

import concourse.bass as bass
import concourse.mybir as mybir
from concourse import bacc
from concourse import dve_ops as _dve_ops
from concourse.bass_utils import run_bass_kernel_spmd
from concourse.dve_spec import AluOp, C0, Spec, Src0, Src1, lower, minn
from concourse.dve_uop import DveOpSpec
from concourse.tile import TileContext

F32 = mybir.dt.float32
FP8 = mybir.dt.float8e5
BIG = 3.0e38
NP_FP8 = ml_dtypes.float8_e5m2


def _make_min2_op():
    """Register a custom DVE op: out = min(in0, in1); accum_out = min(s0, min_k out).

    Stock tensor_tensor_reduce has no ucode behind it on this target (runtime
    INTERNAL error), so the pair-min + free-dim-min-reduce is a custom op.
    """
    name = "MIN2_REDUCE_ANT"
    for existing in _dve_ops.OPS:
        if existing.name == name:
            return existing
    spec = Spec(
        body=minn(Src0, Src1),
        accum=AluOp.MIN,
        accum_init=C0,
        reference=lambda in0, in1, c0, c1, c2: (
            np.minimum(in0, in1),
            np.minimum(
                np.asarray(c0, np.float32).reshape(-1, 1)
                if isinstance(c0, np.ndarray)
                else np.float32(c0),
                np.minimum(in0, in1).min(axis=-1, keepdims=True),
            )
            * np.ones((in0.shape[0], 1), np.float32),
        ),
    )
    opcode = _dve_ops._CUSTOM_DVE_ROW_BASE + len(_dve_ops.OPS)
    shas = {}
    for ver in ("v3", "v4"):
        try:
            tmp = DveOpSpec(
                name=name,
                opcode=opcode,
                uops=lower(spec, ver=ver),
                rd1_en=_dve_ops.has_src1(spec),
            )
            shas[ver] = tmp.sha(ver)
        except Exception:
            pass
    op = _dve_ops.DveOp(name, spec, subdim=False, uops_sha=shas)
    _dve_ops.OPS.append(op)
    _dve_ops.CUSTOM_DVE_SPECS[name] = spec
    _dve_ops._SUB_OPCODE_FOR_NAME[name] = opcode
    return op


MIN2 = _make_min2_op()

# Problem constants (hardcoded per contract)
B = 4
D = 3
M = 8192
N = 8192
N_CORES = 8
M_SHARD = M // 2  # 4096 src points per core

NLEV = 5         # e5m2 split levels per value
LMAX = 4         # keep cross pairs with a+b <= LMAX
_PAIRS = [(a, b) for a in range(NLEV) for b in range(NLEV) if a + b <= LMAX]
K_ROWS = 3 * len(_PAIRS) + 2 * NLEV  # 55
K_SHIP = 56      # shipped contraction rows (1 zero pad row; K>=56 keeps
                 # the warm PE_HAM state once K=128 warm-ups have tripped it)
K_FULL = 128     # warm-up matmul contraction rows (cold HAM only counts
                 # full-K activity)

P = 128          # output partitions per M-tile
MM_N = 512       # matmul output free dim (fp32 PSUM: 1 bank)
PSUM_FD = 1024   # min-reduce operand width (2 PSUM banks)
N_WARMUP = 4     # scratch matmuls to lift PE_HAM out of the cold p-state


def build_nc(m_shard: int = M_SHARD, n: int = N, reps: int = 1) -> bass.Bass:
    assert m_shard % P == 0 and n % (4 * PSUM_FD) == 0
    m_tiles = m_shard // P
    pairs = n // (2 * PSUM_FD)  # min-reduce pairs per M-tile

    nc = bacc.Bacc()
    src = nc.dram_tensor("src", [K_SHIP, m_shard], FP8, kind="ExternalInput")
    dst = nc.dram_tensor("dst", [K_SHIP, n], FP8, kind="ExternalInput")
    out = nc.dram_tensor("out", [P, m_tiles, 4], F32, kind="ExternalOutput")

    with TileContext(nc) as tc:
        with (
            tc.tile_pool(name="big", bufs=1) as big,
            tc.tile_pool(name="scr", bufs=3) as scr,
            tc.tile_pool(name="psum", bufs=4, space="PSUM") as psum,
        ):
            # src is split into the mt=0 slice + the rest, and dst into one
            # tile per pr-chunk: Tile tracks DMA completion per tile, so the
            # first matmuls start as soon as their own small chunks land
            # (not after the full 4.5 MB transfer).
            srcT0 = big.tile([K_SHIP, P], FP8)
            srcT = big.tile([K_SHIP, m_shard - P], FP8)
            dstTs = [
                big.tile([K_SHIP, PSUM_FD], FP8, name=f"dstT{i}")
                for i in range(2 * pairs)
            ]
            # 4 independent 8-M-tile accumulator groups: each group's output
            # DMA depends only on its own tile, so it fires mid-kernel.
            n_grp = max(1, m_tiles // 8)
            mins4g = [
                big.tile([P, min(8, m_tiles), 4], F32, name=f"mins4g{g}")
                for g in range(n_grp)
            ]

            # Input DMAs in consumption order: pr=0's pB half (consumed
            # first, via the staging copy), then pA's half and the mt=0 src
            # slice, then the bulk. All on the sync HWDGE queue (the ScalarE
            # queue is gated behind its ACT_TABLE_LOAD and starts later).
            nc.sync.dma_start(out=dstTs[1], in_=dst[:, PSUM_FD : 2 * PSUM_FD])
            nc.sync.dma_start(out=dstTs[0], in_=dst[:, :PSUM_FD])
            nc.sync.dma_start(out=srcT0, in_=src[:, :P])
            dma_order = [i for p in range(1, pairs) for i in (2 * p + 1, 2 * p)]
            for i in dma_order:
                nc.sync.dma_start(
                    out=dstTs[i],
                    in_=dst[:, i * PSUM_FD : (i + 1) * PSUM_FD],
                )
            nc.sync.dma_start(out=srcT, in_=src[:, P:])

            # PE warm-up: full-K matmuls on zeroed scratch (the PSUM result
            # is never read), issued first so the HAM SHORT window sees a
            # busy array and un-throttles the PE clock before the real
            # stream begins. GpSimd zeroes it (live earliest after preamble).
            warm = big.tile([K_FULL, MM_N], FP8)
            nc.gpsimd.memset(warm, 0.0)
            wps = psum.tile([P, PSUM_FD], F32, tag="ps", name="wps")
            for i in range(N_WARMUP):
                nc.tensor.matmul(
                    wps[:, :MM_N], warm[:, :P], warm, start=True, stop=True
                )

            # --- main loop: 1 M-tile = 128 src points vs all n dst points -
            for mt in [t for _ in range(reps) for t in range(m_tiles)]:
                if mt == 0:
                    lhsT = srcT0[:, :]  # [128, 128]
                else:
                    lhsT = srcT[:, (mt - 1) * P : mt * P]
                for pr in range(pairs):
                    pA = psum.tile([P, PSUM_FD], F32, tag="ps")
                    pB = psum.tile([P, PSUM_FD], F32, tag="ps")
                    # Fill pB FIRST: the ScalarE staging copy is on the
                    # critical PSUM-recycle chain, so it must start as early
                    # as possible; pA's matmuls then overlap the copy.
                    for t, pt in ((1, pB), (0, pA)):
                        dchunk = dstTs[2 * pr + t]
                        for h in range(PSUM_FD // MM_N):
                            n0 = h * MM_N
                            nc.tensor.matmul(
                                pt[:, h * MM_N : (h + 1) * MM_N],
                                lhsT,
                                dchunk[:, n0 : n0 + MM_N],
                                start=True,
                                stop=True,
                            )
                        if t == 1:
                            # ISA: only one non-scalar DVE input may live in
                            # PSUM; the (otherwise idle) ScalarE stages pB
                            # into SBUF right behind pB's matmuls.
                            sB = scr.tile([P, PSUM_FD], F32, tag="cp")
                            nc.scalar.copy(out=sB, in_=pB)
                    ttr_out = scr.tile([P, PSUM_FD], F32, tag="ttr")
                    nc.vector._custom_dve(
                        MIN2,
                        out=ttr_out,
                        in0=pA,
                        in1=sB,
                        s0=BIG,
                        accum_out=mins4g[mt // 8][:, mt % 8, pr : pr + 1],
                    )
                # Stream results out in 8-M-tile groups so the final DMA
                # covers only the last group (host does the tiny 4-way min);
                # the last group streams per-M-tile to shrink the tail.
                if reps == 1:
                    if mt >= m_tiles - 8:
                        nc.sync.dma_start(
                            out=out[:, mt : mt + 1, :],
                            in_=mins4g[mt // 8][:, mt % 8 : mt % 8 + 1, :],
                        )
                    elif mt % 8 == 7:
                        g = mt // 8
                        nc.sync.dma_start(
                            out=out[:, g * 8 : g * 8 + 8, :], in_=mins4g[g]
                        )
            if reps != 1:
                for g in range(n_grp):
                    nc.sync.dma_start(
                        out=out[:, g * 8 : g * 8 + 8, :], in_=mins4g[g]
                    )

    nc.finalize()
    return nc


def _split_levels(x64: np.ndarray, nlev: int = NLEV) -> list[np.ndarray]:
    """Decompose float64 x into nlev fp8e5m2 levels, x ~= sum(levels)."""
    levels = []
    r = x64.copy()
    for _ in range(nlev):
        li = r.astype(np.float32).astype(NP_FP8)
        levels.append(li)
        r = r - li.astype(np.float64)
    return levels


def _prep_operands(src_f32: np.ndarray, dst_f32: np.ndarray) -> tuple[np.ndarray, np.ndarray]:
    """Build the [128, m] stationary and [128, n] moving fp8 operands."""
    m = src_f32.shape[1]
    n = dst_f32.shape[1]
    s64 = src_f32.astype(np.float64)
    d64 = dst_f32.astype(np.float64)
    s_lev = _split_levels(s64)                      # each [3, m]
    d_lev = _split_levels(d64)                      # each [3, n]
    ssq = _split_levels(np.sum(s64 * s64, axis=0))  # each [m]
    dsq = _split_levels(np.sum(d64 * d64, axis=0))  # each [n]

    lhsT = np.zeros((K_SHIP, m), NP_FP8)
    rhs = np.zeros((K_SHIP, n), NP_FP8)
    r = 0
    for a, b in _PAIRS:
        neg2sa = (-2.0 * s_lev[a].astype(np.float64)).astype(NP_FP8)  # exact *2
        lhsT[r : r + 3] = neg2sa
        rhs[r : r + 3] = d_lev[b]
        r += 3
    for a in range(NLEV):
        lhsT[r] = ssq[a]
        rhs[r] = NP_FP8(1.0)
        r += 1
    for b in range(NLEV):
        lhsT[r] = NP_FP8(1.0)
        rhs[r] = dsq[b]
        r += 1
    assert r == K_ROWS
    return lhsT, rhs


_NC_CACHE: dict = {}


def _get_nc(m_shard: int, n: int) -> bass.Bass:
    key = (m_shard, n)
    if key not in _NC_CACHE:
        _NC_CACHE[key] = build_nc(m_shard, n)
    return _NC_CACHE[key]


LAST_RESULTS = None  # test harness can inspect exec_time_ns etc.


def kernel(pc_src: np.ndarray, pc_dst: np.ndarray) -> np.ndarray:
    pc_src = np.ascontiguousarray(np.asarray(pc_src), dtype=np.float32)
    pc_dst = np.ascontiguousarray(np.asarray(pc_dst), dtype=np.float32)
    assert pc_src.shape == (B, D, M) and pc_dst.shape == (B, D, N)

    nc = _get_nc(M_SHARD, N)

    in_maps = []
    for c in range(N_CORES):
        b, h = divmod(c, 2)
        lhsT, rhs = _prep_operands(
            pc_src[b, :, h * M_SHARD : (h + 1) * M_SHARD], pc_dst[b]
        )
        in_maps.append({"src": lhsT, "dst": rhs})

    global LAST_RESULTS
    LAST_RESULTS = run_bass_kernel_spmd(nc, in_maps, core_ids=list(range(N_CORES)))

    # host: O(B*M) postprocess (4-way min + sqrt + mean) per core
    md2 = np.concatenate(
        [
            LAST_RESULTS.results[c]["out"].min(axis=2).T.reshape(-1)
            for c in range(N_CORES)
        ]
    )
    md2 = np.maximum(md2, 0.0)
    dists = np.sqrt(md2, dtype=np.float32)
    return np.asarray(np.mean(dists, dtype=np.float32), dtype=np.float32)


# revision 23
# speedup vs baseline: 1.4225x; 1.4225x over previous
"""Chamfer loss (single-direction) Trainium2 Bass kernel.

Problem: pc_src [B=4, 3, M=8192], pc_dst [B=4, 3, N=8192] (fp32).
  d2[b,m,n] = ||src[b,:,m] - dst[b,:,n]||^2
  out = mean over (b,m) of sqrt(min_n d2[b,m,n])

Sharding: 8 cores = 4 batches x 2 M-halves. Each core handles one batch's
dst [3, 8192] and a 4096-point slice of that batch's src. The min over n is
complete per core; the host concatenates per-core min-d2 vectors and does
the (tiny, O(B*M)) sqrt + mean.

Device algorithm per core -- plain fp8e5m2 multi-level augmented matmul with
the contraction dim zero-padded to K=128:
  Each fp32 value is decomposed into NLEV=6 e5m2 levels x = sum_a x_a with
  |x_a| shrinking ~8x per level, so the reconstruction residual is ~2^-18
  relative. fp8 x fp8 products are exact in the fp32 PSUM accumulator.

  d2 = ||s||^2 + ||d||^2 - 2 s.d expands over level pairs; all pairs with
  a + b <= LMAX=6 are kept (dropped terms ~8^-7):
    cross rows:  lhsT = -2*s_a[coord],  rhs = d_b[coord]   (26 pairs x 3)
    norm rows:   lhsT = ssq_a, rhs = 1  /  lhsT = 1, rhs = dsq_b  (6 + 6)
  = 90 contraction rows, zero-padded to 128. K <= 128 is free on the PE
  (throughput is 1 moving column/cycle regardless of K), and full-K
  matmuls keep the PE_HAM activity monitor warm: partial-K matmuls
  (45 or 90 rows) leave the PE clock-gated at 1.2 GHz (427 ns / 512-col
  matmul) forever, while K=128 un-throttles to 2.4 GHz (216 ns) after
  ~3.5 us. A burst of warm-up matmuls on scratch data runs while the
  input DMAs land so the real stream starts warm.

  The min-reduce runs on the VectorEngine with a custom pair-min op,
  one instruction per pair of [128, 1024] PSUM tiles:
    mins4[:, mt, pr] = min(BIG, min_free(min(psumA, sbufB)))
  (2 distance elements per cycle per lane, both read ports). ScalarE
  (otherwise idle) stages psumB into SBUF -- the ISA allows only one
  non-scalar DVE input in PSUM. The 4 pair results per M-tile land in
  independent accum slots (no serial chain), and one final grouped
  tensor_reduce collapses mins4 [P, m_tiles, 4] -> [P, m_tiles].
"""

import ml_dtypes
import numpy as np

import concourse.bass as bass
import concourse.mybir as mybir
from concourse import bacc
from concourse import dve_ops as _dve_ops
from concourse.bass_utils import run_bass_kernel_spmd
from concourse.dve_spec import AluOp, C0, Spec, Src0, Src1, lower, minn
from concourse.dve_uop import DveOpSpec
from concourse.tile import TileContext

F32 = mybir.dt.float32
FP8 = mybir.dt.float8e5
BIG = 3.0e38
NP_FP8 = ml_dtypes.float8_e5m2


def _make_min2_op():
    """Register a custom DVE op: out = min(in0, in1); accum_out = min(s0, min_k out).

    Stock tensor_tensor_reduce has no ucode behind it on this target (runtime
    INTERNAL error), so the pair-min + free-dim-min-reduce is a custom op.
    """
    name = "MIN2_REDUCE_ANT"
    for existing in _dve_ops.OPS:
        if existing.name == name:
            return existing
    spec = Spec(
        body=minn(Src0, Src1),
        accum=AluOp.MIN,
        accum_init=C0,
        reference=lambda in0, in1, c0, c1, c2: (
            np.minimum(in0, in1),
            np.minimum(
                np.asarray(c0, np.float32).reshape(-1, 1)
                if isinstance(c0, np.ndarray)
                else np.float32(c0),
                np.minimum(in0, in1).min(axis=-1, keepdims=True),
            )
            * np.ones((in0.shape[0], 1), np.float32),
        ),
    )
    opcode = _dve_ops._CUSTOM_DVE_ROW_BASE + len(_dve_ops.OPS)
    shas = {}
    for ver in ("v3", "v4"):
        try:
            tmp = DveOpSpec(
                name=name,
                opcode=opcode,
                uops=lower(spec, ver=ver),
                rd1_en=_dve_ops.has_src1(spec),
            )
            shas[ver] = tmp.sha(ver)
        except Exception:
            pass
    op = _dve_ops.DveOp(name, spec, subdim=False, uops_sha=shas)
    _dve_ops.OPS.append(op)
    _dve_ops.CUSTOM_DVE_SPECS[name] = spec
    _dve_ops._SUB_OPCODE_FOR_NAME[name] = opcode
    return op


MIN2 = _make_min2_op()

# Problem constants (hardcoded per contract)
B = 4
D = 3
M = 8192
N = 8192
N_CORES = 8
M_SHARD = M // 2  # 4096 src points per core

NLEV = 6         # e5m2 split levels per value
LMAX = 6         # keep cross pairs with a+b <= LMAX
_PAIRS = [(a, b) for a in range(NLEV) for b in range(NLEV) if a + b <= LMAX]
K_ROWS = 3 * len(_PAIRS) + 2 * NLEV  # 90
K_SHIP = 96      # shipped contraction rows (6 zero pad rows). HAM-warmth
                 # maintenance needs row-fraction x PE-duty above ~0.5:
                 # K=56 at the kernel's ~76% PE duty re-throttles (235us
                 # measured); K=90-96 maintains, and saves 25% input bytes
                 # vs K=128.
K_FULL = 128     # warm-up matmul contraction rows (cold HAM only counts
                 # full-K activity)

P = 128          # output partitions per M-tile
MM_N = 512       # matmul output free dim (fp32 PSUM: 1 bank)
PSUM_FD = 1024   # min-reduce operand width (2 PSUM banks)
N_WARMUP = 4     # scratch matmuls to lift PE_HAM out of the cold p-state


def build_nc(m_shard: int = M_SHARD, n: int = N, reps: int = 1) -> bass.Bass:
    assert m_shard % P == 0 and n % (4 * PSUM_FD) == 0
    m_tiles = m_shard // P
    pairs = n // (2 * PSUM_FD)  # min-reduce pairs per M-tile

    nc = bacc.Bacc()
    src = nc.dram_tensor("src", [K_SHIP, m_shard], FP8, kind="ExternalInput")
    dst = nc.dram_tensor("dst", [K_SHIP, n], FP8, kind="ExternalInput")
    out = nc.dram_tensor("out", [P, m_tiles, 4], F32, kind="ExternalOutput")

    with TileContext(nc) as tc:
        with (
            tc.tile_pool(name="big", bufs=1) as big,
            tc.tile_pool(name="scr", bufs=3) as scr,
            tc.tile_pool(name="psum", bufs=4, space="PSUM") as psum,
        ):
            # src is split into the mt=0 slice + the rest, and dst into one
            # tile per pr-chunk: Tile tracks DMA completion per tile, so the
            # first matmuls start as soon as their own small chunks land
            # (not after the full 4.5 MB transfer).
            srcT0 = big.tile([K_SHIP, P], FP8)
            srcT = big.tile([K_SHIP, m_shard - P], FP8)
            dstTs = [
                big.tile([K_SHIP, PSUM_FD], FP8, name=f"dstT{i}")
                for i in range(2 * pairs)
            ]
            # 4 independent 8-M-tile accumulator groups: each group's output
            # DMA depends only on its own tile, so it fires mid-kernel.
            n_grp = max(1, m_tiles // 8)
            mins4g = [
                big.tile([P, min(8, m_tiles), 4], F32, name=f"mins4g{g}")
                for g in range(n_grp)
            ]

            # Input DMAs in consumption order: pr=0's pB half (consumed
            # first, via the staging copy), then pA's half and the mt=0 src
            # slice, then the bulk. All on the sync HWDGE queue (the ScalarE
            # queue is gated behind its ACT_TABLE_LOAD and starts later).
            nc.sync.dma_start(out=dstTs[1], in_=dst[:, PSUM_FD : 2 * PSUM_FD])
            nc.sync.dma_start(out=dstTs[0], in_=dst[:, :PSUM_FD])
            nc.sync.dma_start(out=srcT0, in_=src[:, :P])
            dma_order = [i for p in range(1, pairs) for i in (2 * p + 1, 2 * p)]
            for i in dma_order:
                nc.sync.dma_start(
                    out=dstTs[i],
                    in_=dst[:, i * PSUM_FD : (i + 1) * PSUM_FD],
                )
            nc.sync.dma_start(out=srcT, in_=src[:, P:])

            # PE warm-up: full-K matmuls on zeroed scratch (the PSUM result
            # is never read), issued first so the HAM SHORT window sees a
            # busy array and un-throttles the PE clock before the real
            # stream begins. GpSimd zeroes it (live earliest after preamble).
            warm = big.tile([K_FULL, MM_N], FP8)
            nc.gpsimd.memset(warm, 0.0)
            wps = psum.tile([P, PSUM_FD], F32, tag="ps", name="wps")
            for i in range(N_WARMUP):
                nc.tensor.matmul(
                    wps[:, :MM_N], warm[:, :P], warm, start=True, stop=True
                )

            # --- main loop: 1 M-tile = 128 src points vs all n dst points -
            for mt in [t for _ in range(reps) for t in range(m_tiles)]:
                if mt == 0:
                    lhsT = srcT0[:, :]  # [128, 128]
                else:
                    lhsT = srcT[:, (mt - 1) * P : mt * P]
                for pr in range(pairs):
                    pA = psum.tile([P, PSUM_FD], F32, tag="ps")
                    pB = psum.tile([P, PSUM_FD], F32, tag="ps")
                    # Fill pB FIRST: the ScalarE staging copy is on the
                    # critical PSUM-recycle chain, so it must start as early
                    # as possible; pA's matmuls then overlap the copy.
                    for t, pt in ((1, pB), (0, pA)):
                        dchunk = dstTs[2 * pr + t]
                        for h in range(PSUM_FD // MM_N):
                            n0 = h * MM_N
                            nc.tensor.matmul(
                                pt[:, h * MM_N : (h + 1) * MM_N],
                                lhsT,
                                dchunk[:, n0 : n0 + MM_N],
                                start=True,
                                stop=True,
                            )
                        if t == 1:
                            # ISA: only one non-scalar DVE input may live in
                            # PSUM; the (otherwise idle) ScalarE stages pB
                            # into SBUF right behind pB's matmuls.
                            sB = scr.tile([P, PSUM_FD], F32, tag="cp")
                            nc.scalar.copy(out=sB, in_=pB)
                    ttr_out = scr.tile([P, PSUM_FD], F32, tag="ttr")
                    nc.vector._custom_dve(
                        MIN2,
                        out=ttr_out,
                        in0=pA,
                        in1=sB,
                        s0=BIG,
                        accum_out=mins4g[mt // 8][:, mt % 8, pr : pr + 1],
                    )
                # Stream results out in 8-M-tile groups so the final DMA
                # covers only the last group (host does the tiny 4-way min);
                # the last group streams per-M-tile to shrink the tail.
                if reps == 1:
                    if mt >= m_tiles - 8:
                        nc.sync.dma_start(
                            out=out[:, mt : mt + 1, :],
                            in_=mins4g[mt // 8][:, mt % 8 : mt % 8 + 1, :],
                        )
                    elif mt % 8 == 7:
                        g = mt // 8
                        nc.sync.dma_start(
                            out=out[:, g * 8 : g * 8 + 8, :], in_=mins4g[g]
                        )
            if reps != 1:
                for g in range(n_grp):
                    nc.sync.dma_start(
                        out=out[:, g * 8 : g * 8 + 8, :], in_=mins4g[g]
                    )

    nc.finalize()
    return nc


def _split_levels(x64: np.ndarray, nlev: int = NLEV) -> list[np.ndarray]:
    """Decompose float64 x into nlev fp8e5m2 levels, x ~= sum(levels)."""
    levels = []
    r = x64.copy()
    for _ in range(nlev):
        li = r.astype(np.float32).astype(NP_FP8)
        levels.append(li)
        r = r - li.astype(np.float64)
    return levels


def _prep_operands(src_f32: np.ndarray, dst_f32: np.ndarray) -> tuple[np.ndarray, np.ndarray]:
    """Build the [128, m] stationary and [128, n] moving fp8 operands."""
    m = src_f32.shape[1]
    n = dst_f32.shape[1]
    s64 = src_f32.astype(np.float64)
    d64 = dst_f32.astype(np.float64)
    s_lev = _split_levels(s64)                      # each [3, m]
    d_lev = _split_levels(d64)                      # each [3, n]
    ssq = _split_levels(np.sum(s64 * s64, axis=0))  # each [m]
    dsq = _split_levels(np.sum(d64 * d64, axis=0))  # each [n]

    lhsT = np.zeros((K_SHIP, m), NP_FP8)
    rhs = np.zeros((K_SHIP, n), NP_FP8)
    r = 0
    for a, b in _PAIRS:
        neg2sa = (-2.0 * s_lev[a].astype(np.float64)).astype(NP_FP8)  # exact *2
        lhsT[r : r + 3] = neg2sa
        rhs[r : r + 3] = d_lev[b]
        r += 3
    for a in range(NLEV):
        lhsT[r] = ssq[a]
        rhs[r] = NP_FP8(1.0)
        r += 1
    for b in range(NLEV):
        lhsT[r] = NP_FP8(1.0)
        rhs[r] = dsq[b]
        r += 1
    assert r == K_ROWS
    return lhsT, rhs


_NC_CACHE: dict = {}


def _get_nc(m_shard: int, n: int) -> bass.Bass:
    key = (m_shard, n)
    if key not in _NC_CACHE:
        _NC_CACHE[key] = build_nc(m_shard, n)
    return _NC_CACHE[key]


LAST_RESULTS = None  # test harness can inspect exec_time_ns etc.


def kernel(pc_src: np.ndarray, pc_dst: np.ndarray) -> np.ndarray:
    pc_src = np.ascontiguousarray(np.asarray(pc_src), dtype=np.float32)
    pc_dst = np.ascontiguousarray(np.asarray(pc_dst), dtype=np.float32)
    assert pc_src.shape == (B, D, M) and pc_dst.shape == (B, D, N)

    nc = _get_nc(M_SHARD, N)

    in_maps = []
    for c in range(N_CORES):
        b, h = divmod(c, 2)
        lhsT, rhs = _prep_operands(
            pc_src[b, :, h * M_SHARD : (h + 1) * M_SHARD], pc_dst[b]
        )
        in_maps.append({"src": lhsT, "dst": rhs})

    global LAST_RESULTS
    LAST_RESULTS = run_bass_kernel_spmd(nc, in_maps, core_ids=list(range(N_CORES)))

    # host: O(B*M) postprocess (4-way min + sqrt + mean) per core
    md2 = np.concatenate(
        [
            LAST_RESULTS.results[c]["out"].min(axis=2).T.reshape(-1)
            for c in range(N_CORES)
        ]
    )
    md2 = np.maximum(md2, 0.0)
    dists = np.sqrt(md2, dtype=np.float32)
    return np.asarray(np.mean(dists, dtype=np.float32), dtype=np.float32)


# revision 24
# speedup vs baseline: 1.4335x; 1.0077x over previous
"""Chamfer loss (single-direction) Trainium2 Bass kernel.

Problem: pc_src [B=4, 3, M=8192], pc_dst [B=4, 3, N=8192] (fp32).
  d2[b,m,n] = ||src[b,:,m] - dst[b,:,n]||^2
  out = mean over (b,m) of sqrt(min_n d2[b,m,n])

Sharding: 8 cores = 4 batches x 2 M-halves. Each core handles one batch's
dst [3, 8192] and a 4096-point slice of that batch's src. The min over n is
complete per core; the host concatenates per-core min-d2 vectors and does
the (tiny, O(B*M)) sqrt + mean.

Device algorithm per core -- plain fp8e5m2 multi-level augmented matmul with
the contraction dim zero-padded to K=128:
  Each fp32 value is decomposed into NLEV=6 e5m2 levels x = sum_a x_a with
  |x_a| shrinking ~8x per level, so the reconstruction residual is ~2^-18
  relative. fp8 x fp8 products are exact in the fp32 PSUM accumulator.

  d2 = ||s||^2 + ||d||^2 - 2 s.d expands over level pairs; all pairs with
  a + b <= LMAX=6 are kept (dropped terms ~8^-7):
    cross rows:  lhsT = -2*s_a[coord],  rhs = d_b[coord]   (26 pairs x 3)
    norm rows:   lhsT = ssq_a, rhs = 1  /  lhsT = 1, rhs = dsq_b  (6 + 6)
  = 90 contraction rows, zero-padded to 128. K <= 128 is free on the PE
  (throughput is 1 moving column/cycle regardless of K), and full-K
  matmuls keep the PE_HAM activity monitor warm: partial-K matmuls
  (45 or 90 rows) leave the PE clock-gated at 1.2 GHz (427 ns / 512-col
  matmul) forever, while K=128 un-throttles to 2.4 GHz (216 ns) after
  ~3.5 us. A burst of warm-up matmuls on scratch data runs while the
  input DMAs land so the real stream starts warm.

  The min-reduce runs on the VectorEngine with a custom pair-min op,
  one instruction per pair of [128, 1024] PSUM tiles:
    mins4[:, mt, pr] = min(BIG, min_free(min(psumA, sbufB)))
  (2 distance elements per cycle per lane, both read ports). ScalarE
  (otherwise idle) stages psumB into SBUF -- the ISA allows only one
  non-scalar DVE input in PSUM. The 4 pair results per M-tile land in
  independent accum slots (no serial chain), and one final grouped
  tensor_reduce collapses mins4 [P, m_tiles, 4] -> [P, m_tiles].
"""

import ml_dtypes
import numpy as np

import concourse.bass as bass
import concourse.mybir as mybir
from concourse import bacc
from concourse import dve_ops as _dve_ops
from concourse.bass_utils import run_bass_kernel_spmd
from concourse.dve_spec import AluOp, C0, Spec, Src0, Src1, lower, minn
from concourse.dve_uop import DveOpSpec
from concourse.tile import TileContext

F32 = mybir.dt.float32
FP8 = mybir.dt.float8e5
BIG = 3.0e38
NP_FP8 = ml_dtypes.float8_e5m2


def _make_min2_op():
    """Register a custom DVE op: out = min(in0, in1); accum_out = min(s0, min_k out).

    Stock tensor_tensor_reduce has no ucode behind it on this target (runtime
    INTERNAL error), so the pair-min + free-dim-min-reduce is a custom op.
    """
    name = "MIN2_REDUCE_ANT"
    for existing in _dve_ops.OPS:
        if existing.name == name:
            return existing
    spec = Spec(
        body=minn(Src0, Src1),
        accum=AluOp.MIN,
        accum_init=C0,
        reference=lambda in0, in1, c0, c1, c2: (
            np.minimum(in0, in1),
            np.minimum(
                np.asarray(c0, np.float32).reshape(-1, 1)
                if isinstance(c0, np.ndarray)
                else np.float32(c0),
                np.minimum(in0, in1).min(axis=-1, keepdims=True),
            )
            * np.ones((in0.shape[0], 1), np.float32),
        ),
    )
    opcode = _dve_ops._CUSTOM_DVE_ROW_BASE + len(_dve_ops.OPS)
    shas = {}
    for ver in ("v3", "v4"):
        try:
            tmp = DveOpSpec(
                name=name,
                opcode=opcode,
                uops=lower(spec, ver=ver),
                rd1_en=_dve_ops.has_src1(spec),
            )
            shas[ver] = tmp.sha(ver)
        except Exception:
            pass
    op = _dve_ops.DveOp(name, spec, subdim=False, uops_sha=shas)
    _dve_ops.OPS.append(op)
    _dve_ops.CUSTOM_DVE_SPECS[name] = spec
    _dve_ops._SUB_OPCODE_FOR_NAME[name] = opcode
    return op


MIN2 = _make_min2_op()

# Problem constants (hardcoded per contract)
B = 4
D = 3
M = 8192
N = 8192
N_CORES = 8
M_SHARD = M // 2  # 4096 src points per core

NLEV = 6         # e5m2 split levels per value
LMAX = 6         # keep cross pairs with a+b <= LMAX
_PAIRS = [(a, b) for a in range(NLEV) for b in range(NLEV) if a + b <= LMAX]
K_ROWS = 3 * len(_PAIRS) + 2 * NLEV  # 90
K_SHIP = 96      # shipped contraction rows (6 zero pad rows). HAM-warmth
                 # maintenance needs row-fraction x PE-duty above ~0.5:
                 # K=56 at the kernel's ~76% PE duty re-throttles (235us
                 # measured); K=90-96 maintains, and saves 25% input bytes
                 # vs K=128.
K_FULL = 128     # warm-up matmul contraction rows (cold HAM only counts
                 # full-K activity)

P = 128          # output partitions per M-tile
MM_N = 512       # matmul output free dim (fp32 PSUM: 1 bank)
PSUM_FD = 1024   # min-reduce operand width (2 PSUM banks)
N_WARMUP = 9     # scratch matmuls to lift PE_HAM out of the cold p-state;
                 # sized so the burst ends right as the first dst chunk's
                 # DMA lands (a PE idle gap resets the HAM busy window)


def build_nc(m_shard: int = M_SHARD, n: int = N, reps: int = 1) -> bass.Bass:
    assert m_shard % P == 0 and n % (4 * PSUM_FD) == 0
    m_tiles = m_shard // P
    pairs = n // (2 * PSUM_FD)  # min-reduce pairs per M-tile

    nc = bacc.Bacc()
    src = nc.dram_tensor("src", [K_SHIP, m_shard], FP8, kind="ExternalInput")
    dst = nc.dram_tensor("dst", [K_SHIP, n], FP8, kind="ExternalInput")
    out = nc.dram_tensor("out", [P, m_tiles, 4], F32, kind="ExternalOutput")

    with TileContext(nc) as tc:
        with (
            tc.tile_pool(name="big", bufs=1) as big,
            tc.tile_pool(name="scr", bufs=3) as scr,
            tc.tile_pool(name="psum", bufs=4, space="PSUM") as psum,
        ):
            # src is split into the mt=0 slice + the rest, and dst into one
            # tile per pr-chunk: Tile tracks DMA completion per tile, so the
            # first matmuls start as soon as their own small chunks land
            # (not after the full 4.5 MB transfer).
            srcT0 = big.tile([K_SHIP, P], FP8)
            srcT = big.tile([K_SHIP, m_shard - P], FP8)
            dstTs = [
                big.tile([K_SHIP, PSUM_FD], FP8, name=f"dstT{i}")
                for i in range(2 * pairs)
            ]
            # 4 independent 8-M-tile accumulator groups: each group's output
            # DMA depends only on its own tile, so it fires mid-kernel.
            n_grp = max(1, m_tiles // 8)
            mins4g = [
                big.tile([P, min(8, m_tiles), 4], F32, name=f"mins4g{g}")
                for g in range(n_grp)
            ]

            # Input DMAs in consumption order: pr=0's pB half (consumed
            # first, via the staging copy), then pA's half and the mt=0 src
            # slice, then the bulk. All on the sync HWDGE queue (the ScalarE
            # queue is gated behind its ACT_TABLE_LOAD and starts later).
            nc.sync.dma_start(out=dstTs[1], in_=dst[:, PSUM_FD : 2 * PSUM_FD])
            nc.sync.dma_start(out=dstTs[0], in_=dst[:, :PSUM_FD])
            nc.sync.dma_start(out=srcT0, in_=src[:, :P])
            dma_order = [i for p in range(1, pairs) for i in (2 * p + 1, 2 * p)]
            for i in dma_order:
                nc.sync.dma_start(
                    out=dstTs[i],
                    in_=dst[:, i * PSUM_FD : (i + 1) * PSUM_FD],
                )
            nc.sync.dma_start(out=srcT, in_=src[:, P:])

            # PE warm-up: full-K matmuls on zeroed scratch (the PSUM result
            # is never read), issued first so the HAM SHORT window sees a
            # busy array and un-throttles the PE clock before the real
            # stream begins. GpSimd zeroes it (live earliest after preamble).
            warm = big.tile([K_FULL, MM_N], FP8)
            nc.gpsimd.memset(warm, 0.0)
            wps = psum.tile([P, PSUM_FD], F32, tag="ps", name="wps")
            for i in range(N_WARMUP):
                nc.tensor.matmul(
                    wps[:, :MM_N], warm[:, :P], warm, start=True, stop=True
                )

            # --- main loop: 1 M-tile = 128 src points vs all n dst points -
            for mt in [t for _ in range(reps) for t in range(m_tiles)]:
                if mt == 0:
                    lhsT = srcT0[:, :]  # [128, 128]
                else:
                    lhsT = srcT[:, (mt - 1) * P : mt * P]
                for pr in range(pairs):
                    pA = psum.tile([P, PSUM_FD], F32, tag="ps")
                    pB = psum.tile([P, PSUM_FD], F32, tag="ps")
                    # Fill pB FIRST: the ScalarE staging copy is on the
                    # critical PSUM-recycle chain, so it must start as early
                    # as possible; pA's matmuls then overlap the copy.
                    for t, pt in ((1, pB), (0, pA)):
                        dchunk = dstTs[2 * pr + t]
                        for h in range(PSUM_FD // MM_N):
                            n0 = h * MM_N
                            nc.tensor.matmul(
                                pt[:, h * MM_N : (h + 1) * MM_N],
                                lhsT,
                                dchunk[:, n0 : n0 + MM_N],
                                start=True,
                                stop=True,
                            )
                        if t == 1:
                            # ISA: only one non-scalar DVE input may live in
                            # PSUM; the (otherwise idle) ScalarE stages pB
                            # into SBUF right behind pB's matmuls.
                            sB = scr.tile([P, PSUM_FD], F32, tag="cp")
                            nc.scalar.copy(out=sB, in_=pB)
                    ttr_out = scr.tile([P, PSUM_FD], F32, tag="ttr")
                    nc.vector._custom_dve(
                        MIN2,
                        out=ttr_out,
                        in0=pA,
                        in1=sB,
                        s0=BIG,
                        accum_out=mins4g[mt // 8][:, mt % 8, pr : pr + 1],
                    )
                # Stream results out in 8-M-tile groups so the final DMA
                # covers only the last group (host does the tiny 4-way min);
                # the last group streams per-M-tile to shrink the tail.
                if reps == 1:
                    if mt >= m_tiles - 8:
                        nc.sync.dma_start(
                            out=out[:, mt : mt + 1, :],
                            in_=mins4g[mt // 8][:, mt % 8 : mt % 8 + 1, :],
                        )
                    elif mt % 8 == 7:
                        g = mt // 8
                        nc.sync.dma_start(
                            out=out[:, g * 8 : g * 8 + 8, :], in_=mins4g[g]
                        )
            if reps != 1:
                for g in range(n_grp):
                    nc.sync.dma_start(
                        out=out[:, g * 8 : g * 8 + 8, :], in_=mins4g[g]
                    )

    nc.finalize()
    return nc


def _split_levels(x64: np.ndarray, nlev: int = NLEV) -> list[np.ndarray]:
    """Decompose float64 x into nlev fp8e5m2 levels, x ~= sum(levels)."""
    levels = []
    r = x64.copy()
    for _ in range(nlev):
        li = r.astype(np.float32).astype(NP_FP8)
        levels.append(li)
        r = r - li.astype(np.float64)
    return levels


def _prep_operands(src_f32: np.ndarray, dst_f32: np.ndarray) -> tuple[np.ndarray, np.ndarray]:
    """Build the [128, m] stationary and [128, n] moving fp8 operands."""
    m = src_f32.shape[1]
    n = dst_f32.shape[1]
    s64 = src_f32.astype(np.float64)
    d64 = dst_f32.astype(np.float64)
    s_lev = _split_levels(s64)                      # each [3, m]
    d_lev = _split_levels(d64)                      # each [3, n]
    ssq = _split_levels(np.sum(s64 * s64, axis=0))  # each [m]
    dsq = _split_levels(np.sum(d64 * d64, axis=0))  # each [n]

    lhsT = np.zeros((K_SHIP, m), NP_FP8)
    rhs = np.zeros((K_SHIP, n), NP_FP8)
    r = 0
    for a, b in _PAIRS:
        neg2sa = (-2.0 * s_lev[a].astype(np.float64)).astype(NP_FP8)  # exact *2
        lhsT[r : r + 3] = neg2sa
        rhs[r : r + 3] = d_lev[b]
        r += 3
    for a in range(NLEV):
        lhsT[r] = ssq[a]
        rhs[r] = NP_FP8(1.0)
        r += 1
    for b in range(NLEV):
        lhsT[r] = NP_FP8(1.0)
        rhs[r] = dsq[b]
        r += 1
    assert r == K_ROWS
    return lhsT, rhs


_NC_CACHE: dict = {}


def _get_nc(m_shard: int, n: int) -> bass.Bass:
    key = (m_shard, n)
    if key not in _NC_CACHE:
        _NC_CACHE[key] = build_nc(m_shard, n)
    return _NC_CACHE[key]


LAST_RESULTS = None  # test harness can inspect exec_time_ns etc.


def kernel(pc_src: np.ndarray, pc_dst: np.ndarray) -> np.ndarray:
    pc_src = np.ascontiguousarray(np.asarray(pc_src), dtype=np.float32)
    pc_dst = np.ascontiguousarray(np.asarray(pc_dst), dtype=np.float32)
    assert pc_src.shape == (B, D, M) and pc_dst.shape == (B, D, N)

    nc = _get_nc(M_SHARD, N)

    in_maps = []
    for c in range(N_CORES):
        b, h = divmod(c, 2)
        lhsT, rhs = _prep_operands(
            pc_src[b, :, h * M_SHARD : (h + 1) * M_SHARD], pc_dst[b]
        )
        in_maps.append({"src": lhsT, "dst": rhs})

    global LAST_RESULTS
    LAST_RESULTS = run_bass_kernel_spmd(nc, in_maps, core_ids=list(range(N_CORES)))

    # host: O(B*M) postprocess (4-way min + sqrt + mean) per core
    md2 = np.concatenate(
        [
            LAST_RESULTS.results[c]["out"].min(axis=2).T.reshape(-1)
            for c in range(N_CORES)
        ]
    )
    md2 = np.maximum(md2, 0.0)
    dists = np.sqrt(md2, dtype=np.float32)
    return np.asarray(np.mean(dists, dtype=np.float32), dtype=np.float32)
